# revision 2
# baseline (speedup 1.0000x reference)
"""Trainium2 Bass kernel for nn_ConditionalRandomField loss.

Data-parallel over batch (64 sequences/core on 8 cores). Two merged
recursions, each a [128-partition x 64-column] chain with block-diagonal
weights W = diag(E, E^T), E = exp(trans - C):

  chain P (plain): partitions 0:64 run the forward alpha recursion,
      64:128 the backward beta recursion; their meet gives log Z.
  chain G (gold):  the same recursion over one-hot-masked emission tiles;
      its product telescopes to the gold-path score.

Pair-slice u stacks emissions for t=u (fwd) on t=1023-u (bwd).  Inits
consume slice 0 (alpha_0 = exp(start) * m_0, Z_0 = exp(end) * m_1023);
merged step s (1..511) does  state <- (W^T state) * pslice[s];  finals
join the halves through one extra E-matmul.  Running plain and gold as two
independent MM->multiply chains hides each other's PE/DVE round-trip
latency (the wall is the per-step cross-engine latency, not engine work).

Window prep (32 slices): one contiguous DMA per direction (bwd stored
ascending), one-hot is_equal on DVE, one [64,128]->[128,64] PE transpose
per slice per chain, exp on ACT in packs of 8.  Gold tiles are
mP * 128 * oh^T (the 128 growth scale is corrected exactly as 1024*ln 128
at the end).  Per-column rescale every 32 steps: colsum-weights matmul ->
reciprocal -> broadcast matmul -> lagged fold into an upcoming tile, with
the bf16-rounded factors logged and corrected exactly in the finals.

Mask is all ones per the problem spec (fill: ones); asserted host-side.
"""

import numpy as np

import concourse.bass as bass
import concourse.tile as tile
from concourse import bacc, mybir
from concourse.bass_utils import run_bass_kernel_spmd

F32 = mybir.dt.float32
BF16 = mybir.dt.bfloat16
I32 = mybir.dt.int32
Alu = mybir.AluOpType
Act = mybir.ActivationFunctionType

B, T, K = 512, 1024, 64
NCORES = 8
BL = B // NCORES            # 64 sequences per core
CNORM = 4.67                # per-step growth constant, folded into E
BIG = 50.0                  # one-hot mask log-offset
NORM = 32                   # rescale period (steps)
LAG = 16                    # rescale folds into the tile consumed LAG steps later
S = T // 2 - 1              # 511 merged steps
W = 32                      # pair-slices per window
NW = (S + 1) // W           # 16 windows of pair-slices (u = 0..511)
TPK = 8                     # pair-slices per pack
NPK = (S + 1) // TPK        # 64 packs
N_EV = 15                   # rescale events per chain (s = 32, 64, .., 480)


def rep_dim(ap, pos, n):
    """Insert a step-0 (broadcast) dim of size n at position pos of an AP."""
    dims = [list(d) for d in ap.ap]
    dims.insert(pos, [0, n])
    return bass.AP(tensor=ap.tensor, offset=ap.offset, ap=dims)


def rev_t(ap, t_hi, n, t_dim_stride):
    """AP starting at column t_hi walking n steps of -t_dim_stride."""
    dims = [list(ap.ap[0])] + [[-t_dim_stride, n]] + [list(d) for d in ap.ap[2:]]
    return bass.AP(tensor=ap.tensor, offset=ap.offset + t_hi * t_dim_stride,
                   ap=dims)


def build_nc():
    nc = bacc.Bacc("TRN2", target_bir_lowering=False, debug=False)

    x = nc.dram_tensor("x", [BL, T, K], F32, kind="ExternalInput")
    tg = nc.dram_tensor("tg", [BL, T], I32, kind="ExternalInput")
    trans = nc.dram_tensor("trans", [K, K], F32, kind="ExternalInput")
    startc = nc.dram_tensor("startc", [K, 1], F32, kind="ExternalInput")
    endc = nc.dram_tensor("endc", [K, 1], F32, kind="ExternalInput")
    out = nc.dram_tensor("out", [1, 1], F32, kind="ExternalOutput")

    from contextlib import ExitStack
    with tile.TileContext(nc) as tc, ExitStack() as ctx:
        con = ctx.enter_context(tc.tile_pool(name="con", bufs=1))
        xwp = ctx.enter_context(tc.tile_pool(name="xwp", bufs=3))
        ohp = ctx.enter_context(tc.tile_pool(name="ohp", bufs=2))
        mpp = ctx.enter_context(tc.tile_pool(name="mpp", bufs=5))
        mgp = ctx.enter_context(tc.tile_pool(name="mgp", bufs=5))
        stpP = ctx.enter_context(tc.tile_pool(name="stpP", bufs=2))
        stpG = ctx.enter_context(tc.tile_pool(name="stpG", bufs=2))
        evp = ctx.enter_context(tc.tile_pool(name="evp", bufs=2))
        fin = ctx.enter_context(tc.tile_pool(name="fin", bufs=1))
        prp = ctx.enter_context(tc.tile_pool(name="prp", bufs=2, space="PSUM"))
        pgp = ctx.enter_context(tc.tile_pool(name="pgp", bufs=2, space="PSUM"))
        psqP = ctx.enter_context(tc.tile_pool(name="psqP", bufs=1, space="PSUM"))
        psqG = ctx.enter_context(tc.tile_pool(name="psqG", bufs=1, space="PSUM"))
        pmx = ctx.enter_context(tc.tile_pool(name="pmx", bufs=2, space="PSUM"))

        # ---------------- constants ----------------
        tg_i = con.tile([BL, T], I32)
        nc.sync.dma_start(tg_i[:], tg[:])
        tg_b = con.tile([BL, T], BF16)
        nc.vector.tensor_copy(tg_b[:], tg_i[:])

        ident = con.tile([K, K], F32)
        onetile = con.tile([K, K], F32)
        nc.vector.memset(onetile[:], 1.0)
        nc.vector.memset(ident[:], 0.0)
        nc.gpsimd.affine_select(ident[:], onetile[:], pattern=[[-1, K]],
                                compare_op=Alu.is_equal, fill=0.0,
                                base=0, channel_multiplier=1)

        GSC = 128.0   # gold per-step scale; ln(GSC) corrected exactly at the end

        iota_i = con.tile([K, K], I32)
        nc.gpsimd.iota(iota_i[:], pattern=[[1, K]], base=0, channel_multiplier=0)
        iota_b = con.tile([K, K], BF16)
        nc.vector.tensor_copy(iota_b[:], iota_i[:])

        bias_mc = con.tile([K, 1], F32)
        nc.vector.memset(bias_mc[:], -CNORM)

        # E = exp(trans - C), Et = E^T; block weights + final weights in bf16
        tr_t = con.tile([K, K], F32)
        nc.sync.dma_start(tr_t[:], trans[:])
        e_f = con.tile([K, K], F32)
        nc.scalar.activation(e_f[:], tr_t[:], Act.Exp, bias=bias_mc[:])
        e_b = con.tile([K, K], BF16)
        nc.vector.tensor_copy(e_b[:], e_f[:])
        p_et_t = pmx.tile([2 * K, BL], F32, tag="mx")
        p_et = p_et_t[0:K, 0:K]
        nc.tensor.transpose(p_et, e_f[:], ident[:])
        et_f = con.tile([K, K], F32)
        nc.vector.tensor_copy(et_f[:], p_et)
        et_b = con.tile([K, K], BF16)
        nc.vector.tensor_copy(et_b[:], et_f[:])

        wblk = con.tile([2 * K, 2 * K], BF16)
        nc.vector.memset(wblk[:], 0.0)
        nc.vector.tensor_copy(wblk[0:K, 0:K], e_b[:])
        nc.sync.dma_start(wblk[K:2 * K, K:2 * K], et_b[:])

        wfin = con.tile([2 * K, K], BF16)
        nc.vector.memset(wfin[:], 0.0)
        nc.sync.dma_start(wfin[K:2 * K, :], et_b[:])

        # rs2: colsum weights (col 0 = rowsum E on fwd partitions,
        #      col 1 = rowsum Et on bwd partitions)
        rs_f = con.tile([K, 1], F32)
        nc.vector.tensor_reduce(rs_f[:], e_f[:], axis=mybir.AxisListType.X, op=Alu.add)
        rs_b = con.tile([K, 1], F32)
        nc.vector.tensor_reduce(rs_b[:], et_f[:], axis=mybir.AxisListType.X, op=Alu.add)
        rs_fb = con.tile([K, 1], BF16)
        nc.vector.tensor_copy(rs_fb[:], rs_f[:])
        rs_bb = con.tile([K, 1], BF16)
        nc.vector.tensor_copy(rs_bb[:], rs_b[:])
        rs2 = con.tile([2 * K, 2], BF16)
        nc.vector.memset(rs2[:], 0.0)
        nc.vector.tensor_copy(rs2[0:K, 0:1], rs_fb[:])
        nc.sync.dma_start(rs2[K:2 * K, 1:2], rs_bb[:])

        sel2 = con.tile([2, 2 * K], BF16)
        ones_row = con.tile([1, K], BF16)
        nc.vector.memset(ones_row[:], 1.0)
        nc.vector.memset(sel2[:], 0.0)
        nc.vector.memset(sel2[0:1, 0:K], 1.0)
        nc.sync.dma_start(sel2[1:2, K:2 * K], ones_row[:])

        se_col = con.tile([2 * K, 1], F32)
        nc.sync.dma_start(se_col[0:K, :], startc[:])
        nc.sync.dma_start(se_col[K:2 * K, :], endc[:])
        zeros2k = con.tile([2 * K, BL], F32)
        nc.vector.memset(zeros2k[:], 0.0)

        ones64 = con.tile([K, 1], BF16)
        nc.vector.memset(ones64[:], 1.0)
        ones2 = con.tile([2, 1], F32)
        nc.vector.memset(ones2[:], 1.0)

        # per-event bf16-rounded rescale factors, event e in columns [e*BL, (e+1)*BL)
        rP_all = con.tile([2, N_EV * BL], F32)
        rG_all = con.tile([2, N_EV * BL], F32)

        # ---------------- window / pack prep ----------------
        xw_tiles = {}
        oh_tiles = {}
        mP_packs = {}
        mG_packs = {}

        def prep_window(w):
            t0 = w * W
            t_hi = T - 1 - t0          # bwd t for slice v is t_hi - v
            xw = xwp.tile([BL, W, 2, K], F32, tag="xw")
            oh = ohp.tile([BL, W, 2, K], F32, tag="oh")
            nc.sync.dma_start(xw[:, :, 0, :],
                              x[:, t0:t0 + W, :].rearrange("b t k -> b (t k)"))
            nc.sync.dma_start(xw[:, :, 1, :], rev_t(x[:, 0:W, :], t_hi, W, K))
            for c0 in range(0, W, TPK):
                c1 = c0 + TPK
                nc.vector.tensor_tensor(
                    out=oh[:, c0:c1, 0, :], in0=rep_dim(iota_b[:], 1, TPK),
                    in1=rep_dim(tg_b[:, t0 + c0:t0 + c1], 2, K),
                    op=Alu.is_equal)
                nc.vector.tensor_tensor(
                    out=oh[:, c0:c1, 1, :], in0=rep_dim(iota_b[:], 1, TPK),
                    in1=rep_dim(rev_t(tg_b[:, 0:TPK], t_hi - c0, TPK, 1), 2, K),
                    op=Alu.is_equal)
            xw_tiles[w] = xw
            oh_tiles[w] = oh

        pack_state = {}

        def prep_pack_half(p, half):
            w = (p * TPK) // W
            xw, oh = xw_tiles[w], oh_tiles[w]
            if half == 0:
                praw = prp.tile([2 * K, TPK, BL], F32, tag="pr")
                poh = pgp.tile([2 * K, TPK, BL], F32, tag="pg")
                pack_state[p] = (praw, poh)
            else:
                praw, poh = pack_state.pop(p)
            h0 = half * (TPK // 2)
            for i in range(h0, h0 + TPK // 2):
                v = (p * TPK + i) % W
                nc.tensor.transpose(praw[:, i, :],
                                    xw[:, v, :, :].rearrange("b a k -> b (a k)"),
                                    ident[:])
                nc.tensor.transpose(poh[:, i, :],
                                    oh[:, v, :, :].rearrange("b a k -> b (a k)"),
                                    ident[:])
            if half == 1:
                mP = mpp.tile([2 * K, TPK, BL], BF16, tag="mP")
                nc.scalar.activation(mP[:], praw[:], Act.Exp)
                mG = mgp.tile([2 * K, TPK, BL], BF16, tag="mG")
                nc.vector.scalar_tensor_tensor(out=mG[:], in0=mP[:], scalar=GSC,
                                               in1=poh[:], op0=Alu.mult,
                                               op1=Alu.mult)
                mP_packs[p] = mP
                mG_packs[p] = mG

        def prep_pack(p):
            prep_pack_half(p, 0)
            prep_pack_half(p, 1)

        def mslice(packs, u):
            return packs[u // TPK][:, u % TPK, :]

        # ---------------- chain inits ----------------
        se_exp = fin.tile([2 * K, BL], BF16, tag="se_exp")
        nc.scalar.activation(se_exp[:], zeros2k[:], Act.Exp, bias=se_col[:])

        prep_window(0)
        prep_pack(0)

        # inits consume pair-slice 0: alpha_0 = exp(start)*m_0 (fwd half),
        # Z_0 = exp(end)*m_1023 (bwd half); the loop then consumes slice s.
        stateP = stpP.tile([2 * K, BL], BF16, tag="xP")
        nc.vector.tensor_tensor(out=stateP[:], in0=se_exp[:],
                                in1=mslice(mP_packs, 0), op=Alu.mult)
        stateG = stpG.tile([2 * K, BL], BF16, tag="xG")
        nc.vector.tensor_tensor(out=stateG[:], in0=se_exp[:],
                                in1=mslice(mG_packs, 0), op=Alu.mult)

        def event(state, r_all_slice, fold_slice, n):
            s2_t = pmx.tile([2 * K, BL], F32, tag="mx")
            s2 = s2_t[0:2, 0:n]
            nc.tensor.matmul(s2, rs2[:], state[:], start=True, stop=True)
            r2 = evp.tile([2, BL], F32, tag="r2")
            nc.vector.reciprocal(r2[:, 0:n], s2)
            # round to bf16 first so the folded scale and the logged scale match
            r2b = evp.tile([2, BL], BF16, tag="r2b")
            nc.vector.tensor_copy(r2b[:, 0:n], r2[:, 0:n])
            nc.vector.tensor_copy(r_all_slice, r2b[:, 0:n])
            bc = pmx.tile([2 * K, BL], F32, tag="mx")
            nc.tensor.matmul(bc[0:2 * K, 0:n], sel2[:], r2b[:, 0:n],
                             start=True, stop=True)
            nc.vector.tensor_tensor(out=fold_slice, in0=fold_slice,
                                    in1=bc[0:2 * K, 0:n], op=Alu.mult)

        # ---------------- merged chains ----------------
        next_pack = 1
        next_half = 0
        next_win = 1
        for s in range(1, S + 1):
            # issue pack halves at up to one half per 2 steps until ~4 packs ahead
            if s % 2 == 1 and next_pack < NPK and next_pack * TPK <= s + 30:
                prep_pack_half(next_pack, next_half)
                if next_half == 1:
                    next_pack += 1
                next_half = 1 - next_half
            if s == 1 or (s % W == 17 and s // W + 2 < NW):
                prep_window(next_win)
                next_win += 1

            qP = psqP.tile([2 * K, BL], F32, tag="qP")
            nc.tensor.matmul(qP[:], wblk[:], stateP[:], start=True, stop=True)
            stateP = stpP.tile([2 * K, BL], BF16, tag="xP")
            nc.vector.tensor_tensor(out=stateP[:], in0=qP[:],
                                    in1=mslice(mP_packs, s), op=Alu.mult)

            qG = psqG.tile([2 * K, BL], F32, tag="qG")
            nc.tensor.matmul(qG[:], wblk[:], stateG[:], start=True, stop=True)
            stateG = stpG.tile([2 * K, BL], BF16, tag="xG")
            nc.vector.tensor_tensor(out=stateG[:], in0=qG[:],
                                    in1=mslice(mG_packs, s), op=Alu.mult)

            if s % NORM == 0 and s // NORM <= N_EV:
                e = s // NORM - 1
                event(stateP, rP_all[:, e * BL:(e + 1) * BL],
                      mslice(mP_packs, s + LAG), BL)
                event(stateG, rG_all[:, e * BL:(e + 1) * BL],
                      mslice(mG_packs, s + LAG), BL)

        # ---------------- finals ----------------
        def final_chain(states, packs, name):
            u_fin = states[0]
            beta_t = pmx.tile([2 * K, BL], F32, tag="mx")
            beta = beta_t[0:K, :]
            nc.tensor.matmul(beta, wfin[:], u_fin[:], start=True, stop=True)
            wz = fin.tile([K, BL], BF16, tag=f"wz{name}")
            nc.vector.tensor_tensor(out=wz[:], in0=beta, in1=u_fin[0:K, :],
                                    op=Alu.mult)
            z_t = pmx.tile([2 * K, BL], F32, tag="mx")
            z = z_t[0:1, :]
            nc.tensor.matmul(z, ones64[:], wz[:], start=True, stop=True)
            lnz = fin.tile([1, BL], F32, tag=f"lnz{name}")
            nc.scalar.activation(lnz[:], z, Act.Ln)
            return lnz

        lnzP = final_chain([stateP], mP_packs, "P")
        lnzG = final_chain([stateG], mG_packs, "G")

        def rlog_row(r_all, name):
            lnr = fin.tile([2, N_EV * BL], F32, tag=f"lnr{name}")
            nc.scalar.activation(lnr[:], r_all[:], Act.Ln)
            acc = None
            for e in range(N_EV):
                nxt = fin.tile([2, BL], F32, tag=f"ac{name}{e % 2}")
                if acc is None:
                    nc.vector.tensor_copy(nxt[:], lnr[:, 0:BL])
                else:
                    nc.vector.tensor_tensor(out=nxt[:], in0=acc[:],
                                            in1=lnr[:, e * BL:(e + 1) * BL],
                                            op=Alu.add)
                acc = nxt
            row_t = pmx.tile([2 * K, BL], F32, tag="mx")
            row = row_t[0:1, :]
            nc.tensor.matmul(row, ones2[:], acc[:], start=True, stop=True)
            row_sb = fin.tile([1, BL], F32, tag=f"row{name}")
            nc.vector.tensor_copy(row_sb[:], row)
            return row_sb

        rowP = rlog_row(rP_all, "P")
        rowG = rlog_row(rG_all, "G")

        d1 = fin.tile([1, BL], F32)
        nc.vector.tensor_tensor(out=d1[:], in0=lnzG[:], in1=lnzP[:],
                                op=Alu.subtract)
        d2 = fin.tile([1, BL], F32)
        nc.vector.tensor_tensor(out=d2[:], in0=rowG[:], in1=rowP[:],
                                op=Alu.subtract)
        v = fin.tile([1, BL], F32)
        nc.vector.tensor_tensor(out=v[:], in0=d1[:], in1=d2[:], op=Alu.subtract)
        tot = fin.tile([1, 1], F32)
        nc.vector.tensor_reduce(tot[:], v[:], axis=mybir.AxisListType.X, op=Alu.add)
        out_sb = fin.tile([1, 1], F32)
        import math
        nc.vector.tensor_scalar_add(out_sb[:], tot[:], -float(BL * T) * math.log(128.0))
        nc.sync.dma_start(out[:], out_sb[:])

    nc.compile()
    return nc


_NC_CACHE = None


def kernel(**inputs) -> np.ndarray:
    global _NC_CACHE
    logits = np.ascontiguousarray(np.asarray(inputs["inputs"], dtype=np.float32))
    tags = np.asarray(inputs["tags"]).astype(np.int32)
    mask = np.asarray(inputs["mask"])
    trans = np.ascontiguousarray(np.asarray(inputs["transitions"], np.float32))
    start = np.asarray(inputs["start_transitions"], np.float32).reshape(K, 1)
    end = np.asarray(inputs["end_transitions"], np.float32).reshape(K, 1)
    assert mask.min() == 1, "kernel assumes mask of all ones (spec fill=ones)"

    if _NC_CACHE is None:
        _NC_CACHE = build_nc()
    nc = _NC_CACHE

    in_maps = []
    for c in range(NCORES):
        sl = slice(c * BL, (c + 1) * BL)
        in_maps.append({
            "x": np.ascontiguousarray(logits[sl]),
            "tg": np.ascontiguousarray(tags[sl]),
            "trans": trans,
            "startc": start,
            "endc": end,
        })
    res = run_bass_kernel_spmd(nc, in_maps, core_ids=list(range(NCORES)))
    total = np.float64(0.0)
    for c in range(NCORES):
        total += np.float64(res.results[c]["out"][0, 0])
    return np.float32(total)
